# revision 1
# baseline (speedup 1.0000x reference)
"""Device program for CrossAttentionNoGate, head-sharded over 8 TRN2 cores.

Core h computes head h for all 4 batches:
  Q_T[b] [32,2048] = (x_q[b] @ wq_h).T / sqrt(32)   (packed: partitions 32b..)
  K_T[b] [32,2048] = (x_kv[b] @ wk_h).T
  V_aug[b][t] [128,33] = [V rows * mask | mask]     (t = kv tile)
  S_T tile = bias_T (PE identity-inject) + K_T.T @ Q_T   (PSUM accumulate)
  P_T = exp(S_T)  (ACT, PSUM->SBUF, f32r)
  O_aug [33,q] = V_aug.T @ P_T  (numerator rows 0..31, denominator row 32)
  OT = O_aug[:32] * (1/den)     (outer-product broadcast of reciprocal)
AllToAll redistributes OT column blocks; each core projects its 1024 rows:
  out_rows = OT_cols.T @ w_o + b_o
"""
from contextlib import ExitStack

import numpy as np

import concourse.bass as bass
import concourse.tile as tile
from concourse import bacc, mybir

F32 = mybir.dt.float32
F32R = mybir.dt.float32r
AF = mybir.ActivationFunctionType

B, Q, KV, C_Q = 4, 2048, 2048, 256
CH = 32
N_CORES = 8
QC = 512
N_QC = Q // QC        # 4
N_SLAB = KV // 128    # 16
SCALE = 1.0 / np.sqrt(CH)


def build(inject_dve_batches=(), debug_ot=False):
    nc = bacc.Bacc("TRN2", target_bir_lowering=False, debug=False, num_devices=N_CORES)

    x_qT = nc.dram_tensor("x_qt", [B, C_Q, Q], F32R, kind="ExternalInput").ap()
    x_kvT = nc.dram_tensor("x_kvt", [B, C_Q, KV], F32R, kind="ExternalInput").ap()
    wq = nc.dram_tensor("wq_h", [C_Q, CH], F32R, kind="ExternalInput").ap()
    wk = nc.dram_tensor("wk_h", [C_Q, CH], F32R, kind="ExternalInput").ap()
    wv = nc.dram_tensor("wv_h", [C_Q, CH], F32R, kind="ExternalInput").ap()
    bias_t = nc.dram_tensor("bias_t", [KV, Q], F32R, kind="ExternalInput").ap()
    mask_p = nc.dram_tensor("mask_p", [B, 128, N_SLAB], F32, kind="ExternalInput").ap()
    ident = nc.dram_tensor("ident", [128, 128], F32R, kind="ExternalInput").ap()
    ones_m = nc.dram_tensor("ones_m", [128, 128], F32R, kind="ExternalInput").ap()
    w_o = nc.dram_tensor("w_o", [C_Q, C_Q], F32R, kind="ExternalInput").ap()
    b_o_row = nc.dram_tensor("b_o_row", [1, C_Q], F32R, kind="ExternalInput").ap()

    out = nc.dram_tensor("out", [B * Q // N_CORES, C_Q], F32, kind="ExternalOutput").ap()
    if debug_ot:
        ot_dbg = nc.dram_tensor("ot_dbg", [N_CORES, CH, 1024], F32, kind="ExternalOutput").ap()

    with tile.TileContext(nc) as tc, ExitStack() as st:
        constp = st.enter_context(tc.tile_pool(name="const", bufs=1))
        persist = st.enter_context(tc.tile_pool(name="persist", bufs=1))
        dramp = st.enter_context(tc.tile_pool(name="dram", bufs=1, space="DRAM"))

        # ---- constants ----
        id_sb = constp.tile([128, 128], F32R)
        nc.sync.dma_start(id_sb[:], ident[:])
        ones_sb = constp.tile([128, 128], F32R)
        nc.sync.dma_start(ones_sb[:], ones_m[:])
        wq_sb = constp.tile([128, 2 * CH], F32R)
        wk_sb = constp.tile([128, 2 * CH], F32R)
        wv_sb = constp.tile([128, 2 * CH], F32R)
        for cc in range(2):
            nc.sync.dma_start(wq_sb[:, cc * CH:(cc + 1) * CH], wq[cc * 128:(cc + 1) * 128, :])
            nc.sync.dma_start(wk_sb[:, cc * CH:(cc + 1) * CH], wk[cc * 128:(cc + 1) * 128, :])
            nc.sync.dma_start(wv_sb[:, cc * CH:(cc + 1) * CH], wv[cc * 128:(cc + 1) * 128, :])
        mask_sb = constp.tile([128, B * N_SLAB], F32)
        for b in range(B):
            nc.sync.dma_start(mask_sb[:, b * N_SLAB:(b + 1) * N_SLAB], mask_p[b])
        wo_sb = constp.tile([128, 2 * C_Q], F32R)
        for dc in range(2):
            nc.sync.dma_start(wo_sb[:, dc * C_Q:(dc + 1) * C_Q], w_o[dc * 128:(dc + 1) * 128, :])
        bo_sb = constp.tile([1, C_Q], F32R)
        nc.sync.dma_start(bo_sb[:], b_o_row[:])

        # persistent activations
        qt_sb = persist.tile([128, Q], F32R)
        kt_sb = persist.tile([128, Q], F32R)
        vaug_sb = persist.tile([128, B * N_SLAB * 33], F32R)
        ot_a2a = dramp.tile([N_CORES, CH, 1024], F32R)
        ot_recv = dramp.tile([N_CORES, CH, 1024], F32R)

        # ---- projections ----
        with (
            tc.tile_pool(name="proj_in", bufs=2) as proj_in,
            tc.tile_pool(name="proj_ps", bufs=2, space="PSUM") as proj_ps,
        ):
            for b in range(B):
                xq = proj_in.tile([128, 2 * Q], F32R, tag="xq", name=f"xq{b}")
                xkv = proj_in.tile([128, 2 * KV], F32R, tag="xkv", name=f"xkv{b}")
                for cc in range(2):
                    nc.sync.dma_start(xq[:, cc * Q:(cc + 1) * Q],
                                      x_qT[b, cc * 128:(cc + 1) * 128, :])
                    nc.sync.dma_start(xkv[:, cc * KV:(cc + 1) * KV],
                                      x_kvT[b, cc * 128:(cc + 1) * 128, :])
                for qc in range(N_QC):
                    pq = proj_ps.tile([32, QC], F32, tag="pq", name=f"pq{b}_{qc}")
                    pk = proj_ps.tile([32, QC], F32, tag="pk", name=f"pk{b}_{qc}")
                    for cc in range(2):
                        nc.tensor.matmul(
                            pq[:], wq_sb[:, cc * CH:(cc + 1) * CH],
                            xq[:, cc * Q + qc * QC: cc * Q + (qc + 1) * QC],
                            start=(cc == 0), stop=(cc == 1),
                        )
                        nc.tensor.matmul(
                            pk[:], wk_sb[:, cc * CH:(cc + 1) * CH],
                            xkv[:, cc * KV + qc * QC: cc * KV + (qc + 1) * QC],
                            start=(cc == 0), stop=(cc == 1),
                        )
                    tmpq = proj_in.tile([32, QC], F32R, tag="tmpq", name=f"tmpq{b}_{qc}")
                    tmpk = proj_in.tile([32, QC], F32R, tag="tmpk", name=f"tmpk{b}_{qc}")
                    nc.vector.tensor_scalar_mul(tmpq[:], pq[:], SCALE)
                    nc.vector.tensor_copy(tmpk[:], pk[:])
                    # DMA moves rows to partition offset 32*b (engines cannot)
                    nc.sync.dma_start(
                        qt_sb[32 * b:32 * (b + 1), qc * QC:(qc + 1) * QC], tmpq[:])
                    nc.sync.dma_start(
                        kt_sb[32 * b:32 * (b + 1), qc * QC:(qc + 1) * QC], tmpk[:])
                for t in range(N_SLAB):
                    pv = proj_ps.tile([128, CH], F32, tag="pv", name=f"pv{b}_{t}")
                    for cc in range(2):
                        nc.tensor.matmul(
                            pv[:], xkv[:, cc * KV + t * 128: cc * KV + (t + 1) * 128],
                            wv_sb[:, cc * CH:(cc + 1) * CH],
                            start=(cc == 0), stop=(cc == 1),
                        )
                    # col 0 = mask (-> denominator on partition 0), cols 1..32 = V*mask
                    col = (b * N_SLAB + t) * 33
                    midx = b * N_SLAB + t
                    nc.vector.tensor_scalar_mul(
                        vaug_sb[:, col + 1:col + 1 + CH], pv[:], mask_sb[:, midx:midx + 1])
                    nc.vector.tensor_copy(
                        vaug_sb[:, col:col + 1], mask_sb[:, midx:midx + 1])

        # ---- attention main loop ----
        with (
            tc.tile_pool(name="biasp", bufs=20) as biasp,
            tc.tile_pool(name="s_ps", bufs=2, space="PSUM") as s_ps,
            tc.tile_pool(name="o_ps", bufs=1, space="PSUM") as o_ps,
            tc.tile_pool(name="ptile", bufs=3) as ptile,
            tc.tile_pool(name="norm", bufs=2) as normp,
        ):
            for qc in range(N_QC):
                bias_tiles = []
                for t in range(N_SLAB):
                    bt = biasp.tile([128, QC], F32R, tag="bias", name=f"bias_{qc}_{t}")
                    nc.sync.dma_start(
                        bt[:], bias_t[t * 128:(t + 1) * 128, qc * QC:(qc + 1) * QC])
                    bias_tiles.append(bt)
                for pr in range(2):
                    b_lo, b_hi = 2 * pr, 2 * pr + 1
                    o_lo = o_ps.tile([33, QC], F32, tag="opsA", name=f"ops_{qc}_{b_lo}")
                    o_hi = o_ps.tile([33, QC], F32, tag="opsB", name=f"ops_{qc}_{b_hi}")
                    for t in range(N_SLAB):
                        s0 = s_ps.tile([128, QC], F32, tag="s0", name=f"s0_{qc}_{pr}_{t}")
                        s1 = s_ps.tile([128, QC], F32, tag="s1", name=f"s1_{qc}_{pr}_{t}")
                        # b_lo: PE identity-inject of bias, then QK accumulates
                        nc.tensor.matmul(s0[:], id_sb[:], bias_tiles[t][:],
                                         start=True, stop=False)
                        nc.tensor.matmul(
                            s0[:],
                            kt_sb[32 * b_lo:32 * (b_lo + 1), t * 128:(t + 1) * 128],
                            qt_sb[32 * b_lo:32 * (b_lo + 1), qc * QC:(qc + 1) * QC],
                            start=False, stop=True, tile_position=(32 * b_lo, 0))
                        # b_hi: bare QK; bias added in-place on DVE
                        nc.tensor.matmul(
                            s1[:],
                            kt_sb[32 * b_hi:32 * (b_hi + 1), t * 128:(t + 1) * 128],
                            qt_sb[32 * b_hi:32 * (b_hi + 1), qc * QC:(qc + 1) * QC],
                            start=True, stop=True, tile_position=(32 * b_hi, 0))
                        nc.vector.tensor_add(s1[:], s1[:], bias_tiles[t][:].bitcast(F32))
                        p0 = ptile.tile([128, QC], F32R, tag="p0", name=f"p0_{qc}_{pr}_{t}")
                        p1 = ptile.tile([128, QC], F32R, tag="p1", name=f"p1_{qc}_{pr}_{t}")
                        nc.scalar.activation(p0[:], s0[:], AF.Exp)
                        nc.scalar.activation(p1[:], s1[:], AF.Exp)
                        col_lo = (b_lo * N_SLAB + t) * 33
                        col_hi = (b_hi * N_SLAB + t) * 33
                        nc.tensor.matmul(o_lo[:], vaug_sb[:, col_lo:col_lo + 33],
                                         p0[:], start=(t == 0), stop=(t == N_SLAB - 1))
                        nc.tensor.matmul(o_hi[:], vaug_sb[:, col_hi:col_hi + 33],
                                         p1[:], start=(t == 0), stop=(t == N_SLAB - 1))
                    for b, o_psum in ((b_lo, o_lo), (b_hi, o_hi)):
                        recip = normp.tile([1, QC], F32, tag="recip",
                                           name=f"recip_{qc}_{b}")
                        nc.vector.reciprocal_approx_fast(recip[:], o_psum[0:1, :])
                        bcast_sb = normp.tile([33, QC], F32, tag="bcast",
                                              name=f"bcastsb_{qc}_{b}")
                        nc.gpsimd.partition_broadcast(bcast_sb[:], recip[:])
                        ot_tile = normp.tile([33, QC], F32R, tag="ot", name=f"ot_{qc}_{b}")
                        # PSUM reads must start at a 32-aligned partition: split
                        # rows 0-31 (row 0 is den*recip, unused) and row 32.
                        nc.vector.tensor_mul(ot_tile[0:32, :], o_psum[0:32, :],
                                             bcast_sb[0:32, :])
                        nc.vector.tensor_mul(ot_tile[32:33, :], o_psum[32:33, :],
                                             bcast_sb[32:33, :])
                        dest = 2 * b + qc // 2
                        lo = 512 * (qc % 2)
                        nc.sync.dma_start(ot_a2a[dest, :, lo:lo + QC], ot_tile[1:33, :])

        if debug_ot:
            nc.sync.dma_start(ot_dbg[:], ot_a2a[:].bitcast(F32))

        # ---- all-to-all + final projection ----
        nc.gpsimd.collective_compute(
            "AllToAll", mybir.AluOpType.bypass,
            replica_groups=[list(range(N_CORES))],
            ins=[ot_a2a[:]], outs=[ot_recv[:]],
        )
        with (
            tc.tile_pool(name="finp", bufs=2) as finp,
            tc.tile_pool(name="fin_ps", bufs=2, space="PSUM") as fin_ps,
        ):
            otl = finp.tile([128, 2 * 1024], F32R, tag="otl", bufs=1)
            for dc in range(2):
                for j in range(4):
                    s = 4 * dc + j
                    nc.sync.dma_start(
                        otl[32 * j:32 * (j + 1), dc * 1024:(dc + 1) * 1024],
                        ot_recv[s])
            for qt in range(8):
                fp = fin_ps.tile([128, C_Q], F32, tag="fin", name=f"fin{qt}")
                nc.tensor.matmul(fp[:], ones_sb[0:1, :], bo_sb[:],
                                 start=True, stop=False)
                for dc in range(2):
                    nc.tensor.matmul(
                        fp[:], otl[:, dc * 1024 + qt * 128: dc * 1024 + (qt + 1) * 128],
                        wo_sb[:, dc * C_Q:(dc + 1) * C_Q],
                        start=False, stop=(dc == 1))
                fout = finp.tile([128, C_Q], F32, tag="fout", name=f"fout{qt}")
                nc.vector.tensor_copy(fout[:], fp[:])
                nc.sync.dma_start(out[qt * 128:(qt + 1) * 128, :], fout[:])

    nc.compile()
    return nc


def host_inputs(input_q, input_kv, mask, bias, w_q, w_k, w_v, w_o, b_o):
    """Build the 8 per-core input maps from the full problem inputs."""
    xq_t = np.ascontiguousarray(input_q.transpose(0, 2, 1))
    xkv_t = np.ascontiguousarray(input_kv.transpose(0, 2, 1))
    mask_v = np.ascontiguousarray(
        mask.reshape(B, KV).reshape(B, N_SLAB, 128).transpose(0, 2, 1)).astype(np.float32)
    ident = np.eye(128, dtype=np.float32)
    ones = np.ones((128, 128), dtype=np.float32)
    bo_row = np.ascontiguousarray(b_o.reshape(1, C_Q))
    w_o = np.ascontiguousarray(w_o)
    in_maps = []
    for h in range(N_CORES):
        sl = slice(h * CH, (h + 1) * CH)
        in_maps.append({
            "x_qt": xq_t,
            "x_kvt": xkv_t,
            "wq_h": np.ascontiguousarray(w_q[:, sl]),
            "wk_h": np.ascontiguousarray(w_k[:, sl]),
            "wv_h": np.ascontiguousarray(w_v[:, sl]),
            "bias_t": np.ascontiguousarray(bias[0, h].T),
            "mask_p": mask_v,
            "ident": ident,
            "ones_m": ones,
            "w_o": w_o,
            "b_o_row": bo_row,
        })
    return in_maps


def unshard(results):
    return np.concatenate([r["out"] for r in results], axis=0).reshape(B, Q, C_Q)


# ---------------------------------------------------------------------------
# Public entry point: full inputs in, full output out.
# ---------------------------------------------------------------------------
_CACHED_NC = None


def _get_nc():
    global _CACHED_NC
    if _CACHED_NC is None:
        _CACHED_NC = build()
    return _CACHED_NC


def kernel(input_q, input_kv, mask, bias, w_q, w_k, w_v, w_o, b_o,
           trace=False, **trace_kwargs):
    from concourse.bass_utils import run_bass_kernel_spmd
    args = [np.asarray(x, dtype=np.float32) for x in
            (input_q, input_kv, mask, bias, w_q, w_k, w_v, w_o, b_o)]
    in_maps = host_inputs(*args)
    nc = _get_nc()
    res = run_bass_kernel_spmd(nc, in_maps, core_ids=list(range(N_CORES)),
                               trace=trace, **trace_kwargs)
    out = unshard(res.results)
    if trace:
        return out, res
    return out

